# revision 13
# baseline (speedup 1.0000x reference)
"""BiLinearAttention TRN2 kernel: (out, score) = attention(query, key, value, W, mask).

  score = softmax((query @ W) @ key^T + mask)   [softmax over s]
  out   = score @ value

Sharding: 8 NeuronCores, core = (batch b = core//2, query-half h = core%2).
Each core computes a [1024, 2048] score block + [1024, 1024] output block.

Precision strategy: all score-path matmuls run as bf16 hi+lo split operands
with 3 matmuls (hi@hi + lo@hi + hi@lo, fp32 PSUM accumulate) -> ~2^-17
effective operand precision at 3 cycles/row (vs fp32 matmul's 4).  The hi/lo
splits are precomputed on the host (same total bytes as fp32).  The P@V
matmul uses fp32r (hw-rounded 11-bit-mantissa fp32 at full PE rate); V ships
as raw fp32 bits declared f32r.  Exact fp32 PE transposes; exp on ScalarE
(LUT, ~5e-5 relative noise).

Per-core dataflow:
  phase 1: QWt[dk, l] = (Q @ W)^T  (lhsT=W hi/lo, rhs=Q^T hi/lo) bf16x3,
    split back to bf16 hi/lo from PSUM.
  per l-tile (software-pipelined S(i) / softmax(i) / transpose+AV(i-1)):
    S chunks in PSUM (bf16x3) -> rowmax (DVE, negated) -> exp(bias=-max,
    rowsum via accum_out) -> PE-transpose unnormalized E (fp32, exact) ->
    cast f32r -> O = Et@V_f32r, scaled by 1/rowsum on copy-out ->
    P = E/rowsum (ScalarE) -> DMA out.
"""
from contextlib import ExitStack

import ml_dtypes
import numpy as np

import concourse.bass as bass
import concourse.mybir as mybir
import concourse.tile as tile
from concourse import bacc
from concourse.bass_utils import run_bass_kernel_spmd
from concourse.masks import make_identity

F32 = mybir.dt.float32
F32R = mybir.dt.float32r
BF16 = mybir.dt.bfloat16
AF = mybir.ActivationFunctionType
AX = mybir.AxisListType
OP = mybir.AluOpType

N_CORES = 8


def _build_program(L, S, DQ, DK, DV, apply_mask, num_devices=N_CORES):
    CH = 512                      # psum chunk width (fp32 bank)
    LT = L // 128                 # l-tiles
    NCH = S // CH                 # s chunks per score row
    KQ = DQ // 128                # contraction tiles for QW
    KD = DK // 128                # dk tiles (partition tiles of QWt)
    SJ = S // 128                 # s-tiles (contraction tiles for AV)
    LCH = max(1, L // CH)         # phase-1 l chunks
    LCW = min(L, CH)
    OCH = max(1, DV // CH)
    OCW = min(DV, CH)

    nc = bacc.Bacc("TRN2", target_bir_lowering=False, debug=False,
                   num_devices=num_devices)

    qhi_d = nc.dram_tensor("qhi", [DQ, L], BF16, kind="ExternalInput").ap()
    qlo_d = nc.dram_tensor("qlo", [DQ, L], BF16, kind="ExternalInput").ap()
    whi_d = nc.dram_tensor("whi", [DQ, DK], BF16, kind="ExternalInput").ap()
    wlo_d = nc.dram_tensor("wlo", [DQ, DK], BF16, kind="ExternalInput").ap()
    khi_d = nc.dram_tensor("khi", [DK, S], BF16, kind="ExternalInput").ap()
    klo_d = nc.dram_tensor("klo", [DK, S], BF16, kind="ExternalInput").ap()
    vr_d = nc.dram_tensor("vr", [S, DV], F32R, kind="ExternalInput").ap()
    if apply_mask:
        maskf = nc.dram_tensor("maskf", [1, S], F32, kind="ExternalInput").ap()
    p = nc.dram_tensor("p", [L, S], F32, kind="ExternalOutput").ap()
    o = nc.dram_tensor("o", [L, DV], F32, kind="ExternalOutput").ap()

    with ExitStack() as ctx:
        tc = ctx.enter_context(tile.TileContext(nc))

        const = ctx.enter_context(tc.tile_pool(name="const", bufs=1))
        ktp = ctx.enter_context(tc.tile_pool(name="ktp", bufs=1))
        qwtp = ctx.enter_context(tc.tile_pool(name="qwtp", bufs=1))
        vh = ctx.enter_context(tc.tile_pool(name="vh", bufs=1))

        ident = const.tile([128, 128], F32, tag="ident")
        make_identity(nc, ident[:])
        if apply_mask:
            # broadcast mask row to all 128 partitions via K=1 matmul with ones
            mrow = const.tile([1, S], F32, tag="mrow")
            nc.sync.dma_start(mrow[:], maskf[:, :])
            ones = const.tile([1, 128], F32, tag="ones")
            nc.vector.memset(ones[:], 1.0)
            m128 = const.tile([128, S], F32, tag="m128")
            with tc.tile_pool(name="ps_m", bufs=2, space="PSUM") as ps_m:
                for c in range(S // CH):
                    pm = ps_m.tile([128, CH], F32, name=f"pm{c}", tag="pm")
                    nc.tensor.matmul(pm[:], ones[:, :], mrow[:, c * CH:(c + 1) * CH],
                                     start=True, stop=True)
                    nc.vector.tensor_copy(m128[:, c * CH:(c + 1) * CH], pm[:])

        # hi/lo bf16 QWt destination tiles (filled in phase 1)
        qwthi = [qwtp.tile([128, L], BF16, tag=f"qwthi{d}", name=f"qwthi{d}")
                 for d in range(KD)]
        qwtlo = [qwtp.tile([128, L], BF16, tag=f"qwtlo{d}", name=f"qwtlo{d}")
                 for d in range(KD)]
        khi = [ktp.tile([128, S], BF16, tag=f"khi{k}", name=f"khi{k}")
               for k in range(KD)]
        klo = [ktp.tile([128, S], BF16, tag=f"klo{k}", name=f"klo{k}")
               for k in range(KD)]
        vrt = [vh.tile([128, DV], F32R, tag=f"vr{j}", name=f"vr{j}")
               for j in range(SJ)]

        # ---- phase 1: QWt[dk, l] = (Q @ W)^T via bf16x3 ----
        # c-pass structure: one 512-wide l-chunk at a time so all KD dk-tiles
        # accumulate in the 8 PSUM banks and each qT chunk is fetched once.
        with (
            tc.tile_pool(name="whl", bufs=1) as whl,
            tc.tile_pool(name="qhl", bufs=3) as qhl,
            tc.tile_pool(name="ps1", bufs=8, space="PSUM") as ps1,
        ):
            whi = [whl.tile([128, DK], BF16, tag=f"whi{k}", name=f"whi{k}")
                   for k in range(KQ)]
            wlo = [whl.tile([128, DK], BF16, tag=f"wlo{k}", name=f"wlo{k}")
                   for k in range(KQ)]

            assert KD <= 8
            for c in range(LCH):
                csl = slice(c * LCW, (c + 1) * LCW)
                chunks = [ps1.tile([128, LCW], F32, name=f"ps1_{d}_{c}", tag="ps1")
                          for d in range(KD)]
                for k in range(KQ):
                    if c == 0:
                        nc.sync.dma_start(whi[k][:], whi_d[k * 128:(k + 1) * 128, :])
                        nc.sync.dma_start(wlo[k][:], wlo_d[k * 128:(k + 1) * 128, :])
                    qh = qhl.tile([128, LCW], BF16, tag="qh", name=f"qh{c}_{k}")
                    ql = qhl.tile([128, LCW], BF16, tag="ql", name=f"ql{c}_{k}")
                    nc.sync.dma_start(qh[:], qhi_d[k * 128:(k + 1) * 128, csl])
                    nc.sync.dma_start(ql[:], qlo_d[k * 128:(k + 1) * 128, csl])
                    for d in range(KD):
                        dsl = slice(d * 128, (d + 1) * 128)
                        ps = chunks[d][:]
                        nc.tensor.matmul(ps, whi[k][:, dsl], qh[:],
                                         start=(k == 0), stop=False)
                        nc.tensor.matmul(ps, wlo[k][:, dsl], qh[:],
                                         start=False, stop=False)
                        nc.tensor.matmul(ps, whi[k][:, dsl], ql[:],
                                         start=False, stop=(k == KQ - 1))
                    if c == 0 and k == KQ - 1:
                        # bulk loads for later phases queue after phase-1 c0
                        for kk in range(KD):
                            nc.sync.dma_start(khi[kk][:],
                                              khi_d[kk * 128:(kk + 1) * 128, :])
                            nc.sync.dma_start(klo[kk][:],
                                              klo_d[kk * 128:(kk + 1) * 128, :])
                        for j in range(SJ):
                            nc.sync.dma_start(vrt[j][:],
                                              vr_d[j * 128:(j + 1) * 128, :])
                for d in range(KD):
                    nc.vector.tensor_copy(qwthi[d][:, csl], chunks[d][:])
                    nc.vector.tensor_tensor(qwtlo[d][:, csl], chunks[d][:],
                                            qwthi[d][:, csl], op=OP.subtract)

        # ---- phase 2/3, software-pipelined per l-tile ----
        # emission order: S(0), sm(0), [S(i), T/AV(i-1), sm(i)]..., T/AV(last)
        with (
            tc.tile_pool(name="ep", bufs=2) as ep,
            tc.tile_pool(name="ptp", bufs=1) as ptp,
            tc.tile_pool(name="op", bufs=2) as op_pool,
            tc.tile_pool(name="scp", bufs=2) as scp,
            tc.tile_pool(name="ps_s", bufs=4, space="PSUM") as ps_s,
            tc.tile_pool(name="ps_t", bufs=2, space="PSUM") as ps_t,
            tc.tile_pool(name="ps_o", bufs=2, space="PSUM") as ps_o,
        ):
            st_chunks = {}
            st_e = {}
            st_rinv = {}

            def emit_S(i):
                lsl = slice(i * 128, (i + 1) * 128)
                schunks = [ps_s.tile([128, CH], F32, name=f"ssc{i}_{c}", tag="ssc")
                           for c in range(NCH)]
                for k in range(KD):
                    for c in range(NCH):
                        csl = slice(c * CH, (c + 1) * CH)
                        nc.tensor.matmul(schunks[c][:], qwthi[k][:, lsl],
                                         khi[k][:, csl],
                                         start=(k == 0), stop=False)
                        nc.tensor.matmul(schunks[c][:], qwtlo[k][:, lsl],
                                         khi[k][:, csl],
                                         start=False, stop=False)
                        nc.tensor.matmul(schunks[c][:], qwthi[k][:, lsl],
                                         klo[k][:, csl],
                                         start=False, stop=(k == KD - 1))
                st_chunks[i] = schunks

            def emit_softmax(i):
                schunks = st_chunks[i]
                scal = scp.tile([128, 8], F32, tag="scal", name=f"scal{i}")
                for c in range(NCH):
                    nc.vector.reduce_max(scal[:, c:c + 1], schunks[c][:],
                                         axis=AX.X, negate=True)
                nm = scal[:, NCH:NCH + 1]
                if NCH == 1:
                    nc.vector.tensor_copy(nm, scal[:, 0:1])
                else:
                    nc.vector.tensor_tensor(nm, scal[:, 0:1], scal[:, 1:2], op=OP.min)
                    for c in range(2, NCH):
                        nc.vector.tensor_tensor(nm, nm, scal[:, c:c + 1], op=OP.min)

                e = ep.tile([128, S], F32, tag="e", name=f"e{i}")
                sums = scp.tile([128, NCH + 2], F32, tag="sums", name=f"sums{i}")
                for c in range(NCH):
                    nc.scalar.activation(e[:, c * CH:(c + 1) * CH], schunks[c][:],
                                         AF.Exp, bias=nm,
                                         accum_out=sums[:, c:c + 1])
                if apply_mask:
                    # multiplicative mask after exp == additive -inf mask
                    # pre-softmax (invalid only for fully-masked rows)
                    nc.vector.tensor_tensor(e[:], e[:], m128[:], op=OP.mult)
                    nc.vector.reduce_sum(sums[:, NCH:NCH + 1], e[:], axis=AX.X)
                    tot = sums[:, NCH:NCH + 1]
                else:
                    tot = sums[:, NCH:NCH + 1]
                    if NCH == 1:
                        tot = sums[:, 0:1]
                    else:
                        nc.vector.tensor_tensor(tot, sums[:, 0:1], sums[:, 1:2],
                                                op=OP.add)
                        for c in range(2, NCH):
                            nc.vector.tensor_tensor(tot, tot, sums[:, c:c + 1],
                                                    op=OP.add)
                rinv = sums[:, NCH + 1:NCH + 2]
                nc.vector.reciprocal(rinv, tot)
                st_e[i] = e
                st_rinv[i] = rinv

            def emit_TAV(i, last=False):
                lsl = slice(i * 128, (i + 1) * 128)
                e = st_e[i]
                rinv = st_rinv[i]
                if last:
                    # final tile: normalize first (nothing left to overlap)
                    nc.scalar.activation(e[:], e[:], AF.Copy, scale=rinv)
                    nc.sync.dma_start(p[lsl, :], e[:])
                # transpose UNNORMALIZED exp values; fold 1/rowsum into O copy
                ptr = []
                for j in range(SJ):
                    pst = ps_t.tile([128, 128], F32, name=f"pst{i}_{j}", tag="pst")
                    nc.tensor.transpose(pst[:], e[:, j * 128:(j + 1) * 128], ident[:])
                    pr = ptp.tile([128, 128], F32R, tag=f"ptr{j}", name=f"ptr{i}_{j}")
                    nc.vector.tensor_copy(pr[:], pst[:])
                    ptr.append(pr)

                pos = [ps_o.tile([128, OCW], F32, name=f"po{i}_{c}", tag="po")
                       for c in range(OCH)]
                for j in range(SJ):
                    for c in range(OCH):
                        nc.tensor.matmul(pos[c][:], ptr[j][:],
                                         vrt[j][:, c * OCW:(c + 1) * OCW],
                                         start=(j == 0), stop=(j == SJ - 1))
                for c in range(OCH):
                    osl = slice(c * OCW, (c + 1) * OCW)
                    ot = op_pool.tile([128, OCW], F32, tag="ot", name=f"ot{i}_{c}")
                    if last:
                        nc.vector.tensor_copy(ot[:], pos[c][:])
                    else:
                        nc.vector.tensor_scalar_mul(ot[:], pos[c][:], rinv)
                    nc.sync.dma_start(o[lsl, osl], ot[:])
                if not last:
                    # normalize P for the score output (off the PE critical path)
                    nc.scalar.activation(e[:], e[:], AF.Copy, scale=rinv)
                    nc.sync.dma_start(p[lsl, :], e[:])

            emit_S(0)
            emit_softmax(0)
            for i in range(1, LT):
                emit_S(i)
                emit_TAV(i - 1)
                emit_softmax(i)
            emit_TAV(LT - 1, last=True)

    nc.compile()
    return nc


_PROGRAM_CACHE = {}


def _get_program(L, S, DQ, DK, DV, apply_mask):
    key = (L, S, DQ, DK, DV, apply_mask)
    if key not in _PROGRAM_CACHE:
        _PROGRAM_CACHE[key] = _build_program(L, S, DQ, DK, DV, apply_mask)
    return _PROGRAM_CACHE[key]


def _split_hilo(x):
    """bf16 hi/lo decomposition: x ~= hi + lo with ~2^-17 relative residual."""
    hi = x.astype(ml_dtypes.bfloat16)
    lo = (x - hi.astype(np.float32)).astype(ml_dtypes.bfloat16)
    return hi, lo


def _run(query, key, value, W, mask, trace=False):
    n, l, dq = query.shape
    _, s, dk = key.shape
    dv = value.shape[2]
    assert n * 2 == N_CORES and l % (2 * 128) == 0
    L = l // 2

    apply_mask = not bool(np.all(mask))
    nc = _get_program(L, s, dq, dk, dv, apply_mask)

    whi, wlo = _split_hilo(np.ascontiguousarray(W, dtype=np.float32))
    per_batch = {}
    for b in range(n):
        kT = np.ascontiguousarray(key[b].T, dtype=np.float32)
        khi, klo = _split_hilo(kT)
        per_batch[b] = (khi, klo, np.ascontiguousarray(value[b], dtype=np.float32))

    in_maps = []
    for core in range(N_CORES):
        b, h = divmod(core, 2)
        khi, klo, vb = per_batch[b]
        qT = np.ascontiguousarray(query[b, h * L:(h + 1) * L, :].T,
                                  dtype=np.float32)
        qhi, qlo = _split_hilo(qT)
        im = {
            "qhi": qhi, "qlo": qlo,
            "whi": whi, "wlo": wlo,
            "khi": khi, "klo": klo,
            "vr": vb,
        }
        if apply_mask:
            im["maskf"] = np.ascontiguousarray(
                mask[b].astype(np.float32)[None, :])
        in_maps.append(im)

    res = run_bass_kernel_spmd(nc, in_maps, core_ids=list(range(N_CORES)),
                               trace=trace)

    score = np.empty((n, l, s), dtype=np.float32)
    out = np.empty((n, l, dv), dtype=np.float32)
    for core in range(N_CORES):
        b, h = divmod(core, 2)
        score[b, h * L:(h + 1) * L, :] = res.results[core]["p"]
        out[b, h * L:(h + 1) * L, :] = res.results[core]["o"]
    return (out, score), res


def kernel(query, key, value, W, mask):
    (out, score), _ = _run(np.asarray(query), np.asarray(key), np.asarray(value),
                           np.asarray(W), np.asarray(mask))
    return (out, score)


# revision 14
# speedup vs baseline: 1.0617x; 1.0617x over previous
"""BiLinearAttention TRN2 kernel: (out, score) = attention(query, key, value, W, mask).

  score = softmax((query @ W) @ key^T + mask)   [softmax over s]
  out   = score @ value

Sharding: 8 NeuronCores, core = (batch b = core//2, query-half h = core%2).
Each core computes a [1024, 2048] score block + [1024, 1024] output block.

Precision strategy: all score-path matmuls run as bf16 hi+lo split operands
with 3 matmuls (hi@hi + lo@hi + hi@lo, fp32 PSUM accumulate) -> ~2^-17
effective operand precision at 3 cycles/row (vs fp32 matmul's 4).  The hi/lo
splits are precomputed on the host (same total bytes as fp32).  The P@V
matmul uses fp32r (hw-rounded 11-bit-mantissa fp32 at full PE rate); V ships
as raw fp32 bits declared f32r.  Exact fp32 PE transposes; exp on ScalarE
(LUT, ~5e-5 relative noise).

Per-core dataflow:
  phase 1: QWt[dk, l] = (Q @ W)^T  (lhsT=W hi/lo, rhs=Q^T hi/lo) bf16x3,
    split back to bf16 hi/lo from PSUM.
  per l-tile (software-pipelined S(i) / softmax(i) / transpose+AV(i-1)):
    S chunks in PSUM (bf16x3) -> rowmax (DVE, negated) -> exp(bias=-max,
    rowsum via accum_out) -> PE-transpose unnormalized E (fp32, exact) ->
    cast f32r -> O = Et@V_f32r, scaled by 1/rowsum on copy-out ->
    P = E/rowsum (ScalarE) -> DMA out.
"""
from contextlib import ExitStack

import ml_dtypes
import numpy as np

import concourse.bass as bass
import concourse.mybir as mybir
import concourse.tile as tile
from concourse import bacc
from concourse.bass_utils import run_bass_kernel_spmd
from concourse.masks import make_identity

F32 = mybir.dt.float32
F32R = mybir.dt.float32r
BF16 = mybir.dt.bfloat16
AF = mybir.ActivationFunctionType
AX = mybir.AxisListType
OP = mybir.AluOpType

N_CORES = 8


def _build_program(L, S, DQ, DK, DV, apply_mask, num_devices=N_CORES):
    CH = 512                      # psum chunk width (fp32 bank)
    LT = L // 128                 # l-tiles
    NCH = S // CH                 # s chunks per score row
    KQ = DQ // 128                # contraction tiles for QW
    KD = DK // 128                # dk tiles (partition tiles of QWt)
    SJ = S // 128                 # s-tiles (contraction tiles for AV)
    LCH = max(1, L // CH)         # phase-1 l chunks
    LCW = min(L, CH)
    OCH = max(1, DV // CH)
    OCW = min(DV, CH)

    nc = bacc.Bacc("TRN2", target_bir_lowering=False, debug=False,
                   num_devices=num_devices)

    qhi_d = nc.dram_tensor("qhi", [DQ, L], BF16, kind="ExternalInput").ap()
    qlo_d = nc.dram_tensor("qlo", [DQ, L], BF16, kind="ExternalInput").ap()
    whi_d = nc.dram_tensor("whi", [DQ, DK], BF16, kind="ExternalInput").ap()
    wlo_d = nc.dram_tensor("wlo", [DQ, DK], BF16, kind="ExternalInput").ap()
    khi_d = nc.dram_tensor("khi", [DK, S], BF16, kind="ExternalInput").ap()
    klo_d = nc.dram_tensor("klo", [DK, S], BF16, kind="ExternalInput").ap()
    vr_d = nc.dram_tensor("vr", [S, DV], F32R, kind="ExternalInput").ap()
    if apply_mask:
        maskf = nc.dram_tensor("maskf", [1, S], F32, kind="ExternalInput").ap()
    p = nc.dram_tensor("p", [L, S], F32, kind="ExternalOutput").ap()
    o = nc.dram_tensor("o", [L, DV], F32, kind="ExternalOutput").ap()

    with ExitStack() as ctx:
        tc = ctx.enter_context(tile.TileContext(nc))

        const = ctx.enter_context(tc.tile_pool(name="const", bufs=1))
        ktp = ctx.enter_context(tc.tile_pool(name="ktp", bufs=1))
        qwtp = ctx.enter_context(tc.tile_pool(name="qwtp", bufs=1))
        vh = ctx.enter_context(tc.tile_pool(name="vh", bufs=1))

        ident = const.tile([128, 128], F32, tag="ident")
        make_identity(nc, ident[:])
        if apply_mask:
            # broadcast mask row to all 128 partitions via K=1 matmul with ones
            mrow = const.tile([1, S], F32, tag="mrow")
            nc.sync.dma_start(mrow[:], maskf[:, :])
            ones = const.tile([1, 128], F32, tag="ones")
            nc.vector.memset(ones[:], 1.0)
            m128 = const.tile([128, S], F32, tag="m128")
            with tc.tile_pool(name="ps_m", bufs=2, space="PSUM") as ps_m:
                for c in range(S // CH):
                    pm = ps_m.tile([128, CH], F32, name=f"pm{c}", tag="pm")
                    nc.tensor.matmul(pm[:], ones[:, :], mrow[:, c * CH:(c + 1) * CH],
                                     start=True, stop=True)
                    nc.vector.tensor_copy(m128[:, c * CH:(c + 1) * CH], pm[:])

        # hi/lo bf16 QWt destination tiles (filled in phase 1)
        qwthi = [qwtp.tile([128, L], BF16, tag=f"qwthi{d}", name=f"qwthi{d}")
                 for d in range(KD)]
        qwtlo = [qwtp.tile([128, L], BF16, tag=f"qwtlo{d}", name=f"qwtlo{d}")
                 for d in range(KD)]
        khi = [ktp.tile([128, S], BF16, tag=f"khi{k}", name=f"khi{k}")
               for k in range(KD)]
        klo = [ktp.tile([128, S], BF16, tag=f"klo{k}", name=f"klo{k}")
               for k in range(KD)]
        vrt = [vh.tile([128, DV], F32R, tag=f"vr{j}", name=f"vr{j}")
               for j in range(SJ)]

        # ---- phase 1: QWt[dk, l] = (Q @ W)^T via bf16x3 ----
        # c-pass structure: one 512-wide l-chunk at a time so all KD dk-tiles
        # accumulate in the 8 PSUM banks and each qT chunk is fetched once.
        with (
            tc.tile_pool(name="whl", bufs=1) as whl,
            tc.tile_pool(name="qhl", bufs=3) as qhl,
            tc.tile_pool(name="ps1", bufs=8, space="PSUM") as ps1,
        ):
            whi = [whl.tile([128, DK], BF16, tag=f"whi{k}", name=f"whi{k}")
                   for k in range(KQ)]
            wlo = [whl.tile([128, DK], BF16, tag=f"wlo{k}", name=f"wlo{k}")
                   for k in range(KQ)]

            assert KD <= 8
            for c in range(LCH):
                csl = slice(c * LCW, (c + 1) * LCW)
                chunks = [ps1.tile([128, LCW], F32, name=f"ps1_{d}_{c}", tag="ps1")
                          for d in range(KD)]
                for k in range(KQ):
                    if c == 0:
                        nc.sync.dma_start(whi[k][:], whi_d[k * 128:(k + 1) * 128, :])
                        nc.sync.dma_start(wlo[k][:], wlo_d[k * 128:(k + 1) * 128, :])
                    qh = qhl.tile([128, LCW], BF16, tag="qh", name=f"qh{c}_{k}")
                    ql = qhl.tile([128, LCW], BF16, tag="ql", name=f"ql{c}_{k}")
                    nc.sync.dma_start(qh[:], qhi_d[k * 128:(k + 1) * 128, csl])
                    nc.sync.dma_start(ql[:], qlo_d[k * 128:(k + 1) * 128, csl])
                    for d in range(KD):
                        dsl = slice(d * 128, (d + 1) * 128)
                        ps = chunks[d][:]
                        nc.tensor.matmul(ps, whi[k][:, dsl], qh[:],
                                         start=(k == 0), stop=False)
                        nc.tensor.matmul(ps, wlo[k][:, dsl], qh[:],
                                         start=False, stop=False)
                        nc.tensor.matmul(ps, whi[k][:, dsl], ql[:],
                                         start=False, stop=(k == KQ - 1))
                    if c == LCH - 1 and k == KQ - 1:
                        # bulk loads for later phases queue after ALL of
                        # phase 1's qT stream (kT arrives just in time for
                        # S(0); V before the first transpose+AV round)
                        for kk in range(KD):
                            nc.sync.dma_start(khi[kk][:],
                                              khi_d[kk * 128:(kk + 1) * 128, :])
                            nc.sync.dma_start(klo[kk][:],
                                              klo_d[kk * 128:(kk + 1) * 128, :])
                        for j in range(SJ):
                            nc.sync.dma_start(vrt[j][:],
                                              vr_d[j * 128:(j + 1) * 128, :])
                for d in range(KD):
                    nc.vector.tensor_copy(qwthi[d][:, csl], chunks[d][:])
                    nc.vector.tensor_tensor(qwtlo[d][:, csl], chunks[d][:],
                                            qwthi[d][:, csl], op=OP.subtract)

        # ---- phase 2/3, software-pipelined per l-tile ----
        # emission order: S(0), sm(0), [S(i), T/AV(i-1), sm(i)]..., T/AV(last)
        with (
            tc.tile_pool(name="ep", bufs=2) as ep,
            tc.tile_pool(name="ptp", bufs=1) as ptp,
            tc.tile_pool(name="op", bufs=2) as op_pool,
            tc.tile_pool(name="scp", bufs=2) as scp,
            tc.tile_pool(name="ps_s", bufs=4, space="PSUM") as ps_s,
            tc.tile_pool(name="ps_t", bufs=2, space="PSUM") as ps_t,
            tc.tile_pool(name="ps_o", bufs=2, space="PSUM") as ps_o,
        ):
            st_chunks = {}
            st_e = {}
            st_rinv = {}

            def emit_S(i):
                lsl = slice(i * 128, (i + 1) * 128)
                schunks = [ps_s.tile([128, CH], F32, name=f"ssc{i}_{c}", tag="ssc")
                           for c in range(NCH)]
                for k in range(KD):
                    for c in range(NCH):
                        csl = slice(c * CH, (c + 1) * CH)
                        nc.tensor.matmul(schunks[c][:], qwthi[k][:, lsl],
                                         khi[k][:, csl],
                                         start=(k == 0), stop=False)
                        nc.tensor.matmul(schunks[c][:], qwtlo[k][:, lsl],
                                         khi[k][:, csl],
                                         start=False, stop=False)
                        nc.tensor.matmul(schunks[c][:], qwthi[k][:, lsl],
                                         klo[k][:, csl],
                                         start=False, stop=(k == KD - 1))
                st_chunks[i] = schunks

            def emit_softmax(i):
                schunks = st_chunks[i]
                scal = scp.tile([128, 8], F32, tag="scal", name=f"scal{i}")
                for c in range(NCH):
                    nc.vector.reduce_max(scal[:, c:c + 1], schunks[c][:],
                                         axis=AX.X, negate=True)
                nm = scal[:, NCH:NCH + 1]
                if NCH == 1:
                    nc.vector.tensor_copy(nm, scal[:, 0:1])
                else:
                    nc.vector.tensor_tensor(nm, scal[:, 0:1], scal[:, 1:2], op=OP.min)
                    for c in range(2, NCH):
                        nc.vector.tensor_tensor(nm, nm, scal[:, c:c + 1], op=OP.min)

                e = ep.tile([128, S], F32, tag="e", name=f"e{i}")
                sums = scp.tile([128, NCH + 2], F32, tag="sums", name=f"sums{i}")
                for c in range(NCH):
                    nc.scalar.activation(e[:, c * CH:(c + 1) * CH], schunks[c][:],
                                         AF.Exp, bias=nm,
                                         accum_out=sums[:, c:c + 1])
                if apply_mask:
                    # multiplicative mask after exp == additive -inf mask
                    # pre-softmax (invalid only for fully-masked rows)
                    nc.vector.tensor_tensor(e[:], e[:], m128[:], op=OP.mult)
                    nc.vector.reduce_sum(sums[:, NCH:NCH + 1], e[:], axis=AX.X)
                    tot = sums[:, NCH:NCH + 1]
                else:
                    tot = sums[:, NCH:NCH + 1]
                    if NCH == 1:
                        tot = sums[:, 0:1]
                    else:
                        nc.vector.tensor_tensor(tot, sums[:, 0:1], sums[:, 1:2],
                                                op=OP.add)
                        for c in range(2, NCH):
                            nc.vector.tensor_tensor(tot, tot, sums[:, c:c + 1],
                                                    op=OP.add)
                rinv = sums[:, NCH + 1:NCH + 2]
                nc.vector.reciprocal(rinv, tot)
                st_e[i] = e
                st_rinv[i] = rinv

            def emit_TAV(i, last=False):
                lsl = slice(i * 128, (i + 1) * 128)
                e = st_e[i]
                rinv = st_rinv[i]
                if last:
                    # final tile: normalize first (nothing left to overlap)
                    nc.scalar.activation(e[:], e[:], AF.Copy, scale=rinv)
                    nc.sync.dma_start(p[lsl, :], e[:])
                # transpose UNNORMALIZED exp values; fold 1/rowsum into O copy
                ptr = []
                for j in range(SJ):
                    pst = ps_t.tile([128, 128], F32, name=f"pst{i}_{j}", tag="pst")
                    nc.tensor.transpose(pst[:], e[:, j * 128:(j + 1) * 128], ident[:])
                    pr = ptp.tile([128, 128], F32R, tag=f"ptr{j}", name=f"ptr{i}_{j}")
                    nc.vector.tensor_copy(pr[:], pst[:])
                    ptr.append(pr)

                pos = [ps_o.tile([128, OCW], F32, name=f"po{i}_{c}", tag="po")
                       for c in range(OCH)]
                for j in range(SJ):
                    for c in range(OCH):
                        nc.tensor.matmul(pos[c][:], ptr[j][:],
                                         vrt[j][:, c * OCW:(c + 1) * OCW],
                                         start=(j == 0), stop=(j == SJ - 1))
                for c in range(OCH):
                    osl = slice(c * OCW, (c + 1) * OCW)
                    ot = op_pool.tile([128, OCW], F32, tag="ot", name=f"ot{i}_{c}")
                    if last:
                        nc.vector.tensor_copy(ot[:], pos[c][:])
                    else:
                        nc.vector.tensor_scalar_mul(ot[:], pos[c][:], rinv)
                    nc.sync.dma_start(o[lsl, osl], ot[:])
                if not last:
                    # normalize P for the score output (off the PE critical path)
                    nc.scalar.activation(e[:], e[:], AF.Copy, scale=rinv)
                    nc.sync.dma_start(p[lsl, :], e[:])

            emit_S(0)
            emit_softmax(0)
            for i in range(1, LT):
                emit_S(i)
                emit_TAV(i - 1)
                emit_softmax(i)
            emit_TAV(LT - 1, last=True)

    nc.compile()
    return nc


_PROGRAM_CACHE = {}


def _get_program(L, S, DQ, DK, DV, apply_mask):
    key = (L, S, DQ, DK, DV, apply_mask)
    if key not in _PROGRAM_CACHE:
        _PROGRAM_CACHE[key] = _build_program(L, S, DQ, DK, DV, apply_mask)
    return _PROGRAM_CACHE[key]


def _split_hilo(x):
    """bf16 hi/lo decomposition: x ~= hi + lo with ~2^-17 relative residual."""
    hi = x.astype(ml_dtypes.bfloat16)
    lo = (x - hi.astype(np.float32)).astype(ml_dtypes.bfloat16)
    return hi, lo


def _run(query, key, value, W, mask, trace=False):
    n, l, dq = query.shape
    _, s, dk = key.shape
    dv = value.shape[2]
    assert n * 2 == N_CORES and l % (2 * 128) == 0
    L = l // 2

    apply_mask = not bool(np.all(mask))
    nc = _get_program(L, s, dq, dk, dv, apply_mask)

    whi, wlo = _split_hilo(np.ascontiguousarray(W, dtype=np.float32))
    per_batch = {}
    for b in range(n):
        kT = np.ascontiguousarray(key[b].T, dtype=np.float32)
        khi, klo = _split_hilo(kT)
        per_batch[b] = (khi, klo, np.ascontiguousarray(value[b], dtype=np.float32))

    in_maps = []
    for core in range(N_CORES):
        b, h = divmod(core, 2)
        khi, klo, vb = per_batch[b]
        qT = np.ascontiguousarray(query[b, h * L:(h + 1) * L, :].T,
                                  dtype=np.float32)
        qhi, qlo = _split_hilo(qT)
        im = {
            "qhi": qhi, "qlo": qlo,
            "whi": whi, "wlo": wlo,
            "khi": khi, "klo": klo,
            "vr": vb,
        }
        if apply_mask:
            im["maskf"] = np.ascontiguousarray(
                mask[b].astype(np.float32)[None, :])
        in_maps.append(im)

    res = run_bass_kernel_spmd(nc, in_maps, core_ids=list(range(N_CORES)),
                               trace=trace)

    score = np.empty((n, l, s), dtype=np.float32)
    out = np.empty((n, l, dv), dtype=np.float32)
    for core in range(N_CORES):
        b, h = divmod(core, 2)
        score[b, h * L:(h + 1) * L, :] = res.results[core]["p"]
        out[b, h * L:(h + 1) * L, :] = res.results[core]["o"]
    return (out, score), res


def kernel(query, key, value, W, mask):
    (out, score), _ = _run(np.asarray(query), np.asarray(key), np.asarray(value),
                           np.asarray(W), np.asarray(mask))
    return (out, score)
